# revision 8
# baseline (speedup 1.0000x reference)
"""Trainium2 Bass kernel for CrossModalAttentionImproved.

Single-head cross attention + FFN transformer block:
  q = Xq@Wq+bq; k = Xk@Wk+bk; v = Xk@Wv+bv
  attn = softmax(q k^T / sqrt(D)); ctx = attn@v
  out = LN(Xq + ctx@Wo + bo; g1,b1)
  h = gelu(LN(out@W1 + bf1; gf,bf))
  y = LN(out + h@W2 + bf2; g2,b2)

Sharding: data-parallel over batch. B=16 across 8 cores x 2 sequential
NEFF runs (one batch element per core per run). Params replicated.

Layout strategy (all matmuls bf16, fp32 PSUM accumulate):
  - host pre-transposes Xq/Xk to [D,N] bf16 so projections contract d on
    partitions with no on-chip input transpose
  - QT/KT produced transposed [D,N]; V natural [N,D] with a ones column
  - scoresT[k,q] = KT.T@QT per k-tile; exp on ACT (scale=1/sqrt(D) folded)
  - ctx[q,d+1] = eT.T @ [V|1]: softmax sums land as a per-partition column
    -> reciprocal + tensor_scalar normalize at PSUM eviction
  - ctx PE-transposed -> Wo -> residual+LN (rsqrt via DVE bit-trick Newton,
    keeps ACT tables to {exp, gelu} only)
  - FFN1 natural out; LN+GELU fused into a single ACT op (scale/bias)
  - h bounced through DRAM with DMA-transpose (2-byte xbar) for FFN2
"""

import sys

if '/opt/trn_rl_repo' not in sys.path:
    sys.path.insert(0, '/opt/trn_rl_repo')

import math
from contextlib import ExitStack

import numpy as np
import ml_dtypes

import concourse.bass as bass
import concourse.tile as tile
from concourse import bacc, mybir
from concourse import bass2jax
from concourse.masks import make_identity

F32 = mybir.dt.float32
BF16 = mybir.dt.bfloat16
U32 = mybir.dt.uint32
Alu = mybir.AluOpType
Act = mybir.ActivationFunctionType

EPS = 1e-5
P = 128


# ---------------------------------------------------------------------------
# device program
# ---------------------------------------------------------------------------

def build_program(N=2048, D=768, H=3072, QB=512,
                  nontrivial=frozenset(), mm_dt=BF16):
    """Build + compile the per-core single-batch-element program.

    nontrivial: subset of {bv, bo, bf1, bf2, g1b1, gfbf, g2b2} naming the
    affine params that are not identity and need real ops emitted.
    """
    DC = D // P          # d chunks (6)
    HC = H // P          # h chunks (24)
    RT = N // P          # row tiles (16)
    NB = N // QB         # q blocks (4)
    SB = QB // P         # subtiles per block (4)
    F1N = min(512, H)    # FFN1 n-chunk width
    F1C = H // F1N       # FFN1 n-chunks (6)
    scale = 1.0 / math.sqrt(D)

    def slices(total):
        """Split [0,total) into <=512-wide psum-bank-sized slices."""
        out, lo = [], 0
        while lo < total:
            hi = min(lo + 512, total)
            out.append((lo, hi))
            lo = hi
        return out

    D_SL = slices(D)          # [(0,512),(512,768)]
    D1_SL = slices(D + 1)     # [(0,512),(512,769)]

    nc = bacc.Bacc("TRN2", target_bir_lowering=False, debug=False,
                   num_devices=8)

    # ---- DRAM I/O -----------------------------------------------------
    d_xqT = nc.dram_tensor("xqT", [D, N], mm_dt, kind="ExternalInput")
    d_xkT = nc.dram_tensor("xkT", [D, N], mm_dt, kind="ExternalInput")
    d_xq = nc.dram_tensor("xq", [N, D], F32, kind="ExternalInput")
    d_wq = nc.dram_tensor("wq", [D, D], mm_dt, kind="ExternalInput")
    d_wk = nc.dram_tensor("wk", [D, D], mm_dt, kind="ExternalInput")
    d_wv = nc.dram_tensor("wv", [D, D], mm_dt, kind="ExternalInput")
    d_wo = nc.dram_tensor("wo", [D, D], mm_dt, kind="ExternalInput")
    d_w1 = nc.dram_tensor("w1", [D, H], mm_dt, kind="ExternalInput")
    d_w2 = nc.dram_tensor("w2", [H, D], mm_dt, kind="ExternalInput")
    d_bq = nc.dram_tensor("bq", [D], F32, kind="ExternalInput")
    d_bk = nc.dram_tensor("bk", [D], F32, kind="ExternalInput")
    dram_aff = {}
    for nm, sz in (("bv", D), ("bo", D), ("bf1", H), ("bf2", D)):
        if nm in nontrivial:
            dram_aff[nm] = nc.dram_tensor(nm, [sz], mm_dt, kind="ExternalInput")
    for nm, sz in (("g1b1", D), ("gfbf", H), ("g2b2", D)):
        if nm in nontrivial:
            dram_aff[nm + "_g"] = nc.dram_tensor(nm + "_g", [sz], F32,
                                                 kind="ExternalInput")
            dram_aff[nm + "_b"] = nc.dram_tensor(nm + "_b", [sz], F32,
                                                 kind="ExternalInput")
    d_y = nc.dram_tensor("y", [N, D], F32, kind="ExternalOutput")
    # internal scratch
    d_outf = nc.dram_tensor("out_f32", [N, D], F32)
    d_outb = nc.dram_tensor("out_b16", [N, D], mm_dt)
    d_h = nc.dram_tensor("h_b16", [N, H], mm_dt)

    # bn_stats subgroup sizes
    bn_d = math.gcd(512, D)      # 256 for 768
    bn_dn = D // bn_d

    def emit_rsqrt(pool, nc, var_ap, tag):
        """rstd[P,1] f32 = 1/sqrt(var+EPS), DVE only (no ACT tables).

        Quake-style bit trick seed + 2 Newton iterations (~5e-6 rel err).
        """
        ve = pool.tile([P, 1], F32, tag=f"rs_ve_{tag}", bufs=2)
        nc.vector.tensor_scalar_add(ve, var_ap, EPS)
        y = pool.tile([P, 1], F32, tag=f"rs_y_{tag}", bufs=2)
        # y_bits = 0x5f3759df - (ve_bits >> 1)  ==  ~(ve_bits>>1) - 0xA0C8A620
        nc.vector.tensor_scalar(
            out=y.bitcast(U32), in0=ve.bitcast(U32),
            scalar1=1, scalar2=0xFFFFFFFF,
            op0=Alu.logical_shift_right, op1=Alu.bitwise_xor)
        nc.vector.tensor_scalar(
            out=y.bitcast(U32), in0=y.bitcast(U32),
            scalar1=0xA0C8A620, scalar2=None, op0=Alu.subtract)
        t = pool.tile([P, 1], F32, tag=f"rs_t_{tag}", bufs=2)
        for _ in range(2):
            nc.vector.tensor_mul(t, y, y)            # y^2
            nc.vector.tensor_mul(t, t, ve)           # v*y^2
            nc.vector.tensor_scalar(out=t, in0=t, scalar1=-0.5, scalar2=1.5,
                                    op0=Alu.mult, op1=Alu.add)
            nc.vector.tensor_mul(y, y, t)
        return y

    def load_rep(pool, nc, dram, sz, tag):
        """Broadcast a [sz] dram vector across partitions -> [P, sz] tile."""
        t = pool.tile([P, sz], dram.dtype, tag=tag, bufs=1)
        ap = dram.ap()
        bcast = bass.AP(tensor=ap.tensor, offset=ap.offset,
                        ap=[[0, P]] + list(ap.ap))
        nc.gpsimd.dma_start(out=t, in_=bcast)
        return t

    with tile.TileContext(nc) as tc, ExitStack() as octx:
        pers = octx.enter_context(tc.tile_pool(name="pers", bufs=1))
        ident = pers.tile([P, P], mm_dt)
        make_identity(nc, ident)
        # per-dout-chunk bias columns for QT/KT eviction (free via ACT bias)
        bq_sb = pers.tile([P, DC], F32)
        nc.sync.dma_start(out=bq_sb, in_=d_bq.ap().rearrange("(c p) -> p c", p=P))
        bk_sb = pers.tile([P, DC], F32)
        nc.sync.dma_start(out=bk_sb, in_=d_bk.ap().rearrange("(c p) -> p c", p=P))
        ones_row = None
        if any(k in nontrivial for k in ("bv", "bo", "bf1", "bf2")):
            ones_row = pers.tile([1, P], mm_dt)
            nc.vector.memset(ones_row, 1.0)
        bias_rows = {}
        for nm in ("bv", "bo", "bf1", "bf2"):
            if nm in nontrivial:
                sz = H if nm == "bf1" else D
                t = pers.tile([1, sz], mm_dt, tag=f"brow_{nm}")
                nc.sync.dma_start(out=t, in_=dram_aff[nm].ap().rearrange(
                    "(o f) -> o f", o=1))
                bias_rows[nm] = t
        gain_reps = {}
        for nm in ("g1b1", "gfbf", "g2b2"):
            if nm in nontrivial:
                sz = H if nm == "gfbf" else D
                gain_reps[nm + "_g"] = load_rep(pers, nc, dram_aff[nm + "_g"],
                                                sz, f"grep_{nm}")
                gain_reps[nm + "_b"] = load_rep(pers, nc, dram_aff[nm + "_b"],
                                                sz, f"brep_{nm}")

        def add_bias_row(psum_ap, nm, lo, hi):
            """Accumulate broadcast bias row into psum via K=1 matmul."""
            nc.tensor.matmul(psum_ap[:, lo:hi], lhsT=ones_row,
                             rhs=bias_rows[nm][:, lo:hi],
                             start=False, stop=True)

        def post_ln_affine(nc, buf, nm):
            if nm in nontrivial:
                nc.vector.tensor_mul(buf, buf, gain_reps[nm + "_g"])
                nc.vector.tensor_add(buf, buf, gain_reps[nm + "_b"])

        # =============== Phases A+B share the Q/K/V tiles ===============
        ab_ctx = octx.enter_context(ExitStack())
        qkv = ab_ctx.enter_context(tc.tile_pool(name="qkv", bufs=1))
        qT = qkv.tile([P, DC, N], mm_dt)
        kT = qkv.tile([P, DC, N], mm_dt)
        v = qkv.tile([P, RT, D + 8], mm_dt)

        with ExitStack() as ctx:
            pa = ctx.enter_context(tc.tile_pool(name="pa", bufs=1))
            psA = ctx.enter_context(tc.tile_pool(name="psA", bufs=3,
                                                 space="PSUM"))
            psV = ctx.enter_context(tc.tile_pool(name="psV", bufs=2,
                                                 space="PSUM"))
            xqT = pa.tile([P, DC, N], mm_dt)
            nc.sync.dma_start(out=xqT, in_=d_xqT.ap().rearrange(
                "(c p) n -> p c n", p=P))
            xkT = pa.tile([P, DC, N], mm_dt)
            nc.sync.dma_start(out=xkT, in_=d_xkT.ap().rearrange(
                "(c p) n -> p c n", p=P))
            wq = pa.tile([P, DC, D], mm_dt)
            nc.sync.dma_start(out=wq, in_=d_wq.ap().rearrange(
                "(c p) f -> p c f", p=P))
            wk = pa.tile([P, DC, D], mm_dt)
            nc.sync.dma_start(out=wk, in_=d_wk.ap().rearrange(
                "(c p) f -> p c f", p=P))
            wv = pa.tile([P, DC, D], mm_dt)
            nc.sync.dma_start(out=wv, in_=d_wv.ap().rearrange(
                "(c p) f -> p c f", p=P))

            # QT/KT: [dout_chunk, rows]; lhsT=W chunk, rhs=xT chunk
            for dst, w_sb, b_sb, xT in ((qT, wq, bq_sb, xqT),
                                        (kT, wk, bk_sb, xkT)):
                for m in range(DC):
                    for rb in range(N // QB):
                        ps = psA.tile([P, QB], F32, tag="psA")
                        for k in range(DC):
                            nc.tensor.matmul(
                                ps, lhsT=w_sb[:, k, m * P:(m + 1) * P],
                                rhs=xT[:, k, rb * QB:(rb + 1) * QB],
                                start=(k == 0), stop=(k == DC - 1))
                        nc.scalar.activation(
                            out=dst[:, m, rb * QB:(rb + 1) * QB], in_=ps,
                            func=Act.Identity, bias=b_sb[:, m:m + 1])
            # V natural: lhsT = xkT chunk (M=rows), rhs = Wv
            for rt in range(RT):
                ps = psV.tile([P, D], F32, tag="psV")
                for lo, hi in D_SL:
                    for k in range(DC):
                        nc.tensor.matmul(
                            ps[:, lo:hi],
                            lhsT=xkT[:, k, rt * P:(rt + 1) * P],
                            rhs=wv[:, k, lo:hi],
                            start=(k == 0),
                            stop=(k == DC - 1 and "bv" not in nontrivial))
                    if "bv" in nontrivial:
                        add_bias_row(ps, "bv", lo, hi)
                nc.scalar.copy(v[:, rt, :D], ps)
                nc.vector.memset(v[:, rt, D:D + 1], 1.0)

        # =============== Phase B: attention + LN1 =======================
        with ExitStack() as ctx:
            pb = ctx.enter_context(tc.tile_pool(name="pb", bufs=1))
            psS = ctx.enter_context(tc.tile_pool(name="psS", bufs=2,
                                                 space="PSUM"))
            psC = ctx.enter_context(tc.tile_pool(name="psC", bufs=1,
                                                 space="PSUM"))
            psT = ctx.enter_context(tc.tile_pool(name="psT", bufs=1,
                                                 space="PSUM"))
            psW = ctx.enter_context(tc.tile_pool(name="psW", bufs=1,
                                                 space="PSUM"))
            wo = pb.tile([P, DC, D], mm_dt)
            nc.sync.dma_start(out=wo, in_=d_wo.ap().rearrange(
                "(c p) f -> p c f", p=P))

            for qb in range(NB):
                eT = pb.tile([P, RT, QB], mm_dt, tag="eT", bufs=2)
                for kt in range(RT):
                    ps = psS.tile([P, QB], F32, tag="psS")
                    for k in range(DC):
                        nc.tensor.matmul(
                            ps, lhsT=kT[:, k, kt * P:(kt + 1) * P],
                            rhs=qT[:, k, qb * QB:(qb + 1) * QB],
                            start=(k == 0), stop=(k == DC - 1))
                    nc.scalar.activation(out=eT[:, kt, :], in_=ps,
                                         func=Act.Exp, scale=scale)
                for s in range(SB):
                    qs = qb * SB + s          # global q subtile
                    ps = psC.tile([P, D + 1], F32, tag="psC")
                    for lo, hi in D1_SL:
                        for kt in range(RT):
                            nc.tensor.matmul(
                                ps[:, lo:hi],
                                lhsT=eT[:, kt, s * P:(s + 1) * P],
                                rhs=v[:, kt, lo:hi],
                                start=(kt == 0), stop=(kt == RT - 1))
                    recip = pb.tile([P, 1], F32, tag="recip", bufs=2)
                    nc.vector.reciprocal(recip, ps[:, D:D + 1])
                    ctxt = pb.tile([P, D], mm_dt, tag="ctx", bufs=2)
                    nc.vector.tensor_scalar_mul(ctxt, ps[:, :D], recip)
                    ctxT = pb.tile([P, DC, P], mm_dt, tag="ctxT", bufs=2)
                    for j in range(DC):
                        pst = psT.tile([P, P], mm_dt, tag="psT")
                        nc.tensor.transpose(pst, ctxt[:, j * P:(j + 1) * P],
                                            ident)
                        nc.scalar.copy(ctxT[:, j, :], pst)
                    ps_a = psW.tile([P, D], F32, tag="psW")
                    for lo, hi in D_SL:
                        for j in range(DC):
                            nc.tensor.matmul(
                                ps_a[:, lo:hi], lhsT=ctxT[:, j, :],
                                rhs=wo[:, j, lo:hi], start=(j == 0),
                                stop=(j == DC - 1 and "bo" not in nontrivial))
                        if "bo" in nontrivial:
                            add_bias_row(ps_a, "bo", lo, hi)
                    xq_t = pb.tile([P, D], F32, tag="xq", bufs=3)
                    nc.sync.dma_start(out=xq_t,
                                      in_=d_xq.ap()[qs * P:(qs + 1) * P, :])
                    r_t = pb.tile([P, D], F32, tag="r", bufs=2)
                    nc.vector.tensor_add(r_t, ps_a, xq_t)
                    st = pb.tile([P, bn_dn, 6], F32, tag="st1", bufs=2)
                    for g in range(bn_dn):
                        nc.vector.bn_stats(st[:, g, :],
                                           r_t[:, g * bn_d:(g + 1) * bn_d])
                    mv = pb.tile([P, 2], F32, tag="mv1", bufs=2)
                    nc.vector.bn_aggr(mv, st)
                    rstd = emit_rsqrt(pb, nc, mv[:, 1:2], "ln1")
                    nmr = pb.tile([P, 1], F32, tag="nmr1", bufs=2)
                    nc.vector.tensor_scalar(out=nmr, in0=mv[:, 0:1],
                                            scalar1=rstd, scalar2=-1.0,
                                            op0=Alu.mult, op1=Alu.mult)
                    out_t = pb.tile([P, D], F32, tag="out", bufs=2)
                    nc.vector.tensor_scalar(out=out_t, in0=r_t, scalar1=rstd,
                                            scalar2=nmr, op0=Alu.mult,
                                            op1=Alu.add)
                    post_ln_affine(nc, out_t, "g1b1")
                    nc.sync.dma_start(out=d_outf.ap()[qs * P:(qs + 1) * P, :],
                                      in_=out_t)
                    # cast f32 -> bf16 in flight (SWDGE)
                    nc.gpsimd.dma_start(
                        out=d_outb.ap()[qs * P:(qs + 1) * P, :], in_=out_t)

        ab_ctx.close()  # free Q/K/V + attention SBUF before FFN

        # =============== Phase C: FFN + LN2/LN3 =========================
        with ExitStack() as ctx:
            pc = ctx.enter_context(tc.tile_pool(name="pc", bufs=1))
            psH = ctx.enter_context(tc.tile_pool(name="psH", bufs=3,
                                                 space="PSUM"))
            psF = ctx.enter_context(tc.tile_pool(name="psF", bufs=2,
                                                 space="PSUM"))
            w1 = pc.tile([P, DC, H], mm_dt)
            nc.sync.dma_start(out=w1, in_=d_w1.ap().rearrange(
                "(c p) f -> p c f", p=P))
            w2 = pc.tile([P, HC, D], mm_dt)
            nc.sync.dma_start(out=w2, in_=d_w2.ap().rearrange(
                "(c p) f -> p c f", p=P))
            outT = pc.tile([P, DC, N], mm_dt)
            for j in range(DC):
                nc.sync.dma_start(out=outT[:, j, :],
                                  in_=d_outb.ap()[:, j * P:(j + 1) * P],
                                  transpose=True)

            for t in range(RT):
                hpre = pc.tile([P, H], mm_dt, tag="hpre", bufs=2)
                st = pc.tile([P, F1C, 6], F32, tag="st2", bufs=2)
                for n in range(F1C):
                    ps = psH.tile([P, F1N], F32, tag="psH")
                    for j in range(DC):
                        nc.tensor.matmul(
                            ps, lhsT=outT[:, j, t * P:(t + 1) * P],
                            rhs=w1[:, j, n * F1N:(n + 1) * F1N],
                            start=(j == 0),
                            stop=(j == DC - 1 and "bf1" not in nontrivial))
                    if "bf1" in nontrivial:
                        add_bias_row(ps, "bf1", n * F1N, (n + 1) * F1N)
                    nc.vector.bn_stats(st[:, n, :], ps)
                    nc.scalar.copy(hpre[:, n * F1N:(n + 1) * F1N], ps)
                mv = pc.tile([P, 2], F32, tag="mv2", bufs=2)
                nc.vector.bn_aggr(mv, st)
                rstd = emit_rsqrt(pc, nc, mv[:, 1:2], "ln2")
                nmr = pc.tile([P, 1], F32, tag="nmr2", bufs=2)
                nc.vector.tensor_scalar(out=nmr, in0=mv[:, 0:1], scalar1=rstd,
                                        scalar2=-1.0, op0=Alu.mult,
                                        op1=Alu.mult)
                h_t = pc.tile([P, H], mm_dt, tag="h", bufs=2)
                if "gfbf" in nontrivial:
                    tmp = pc.tile([P, H], F32, tag="lnh", bufs=2)
                    nc.vector.tensor_scalar(out=tmp, in0=hpre, scalar1=rstd,
                                            scalar2=nmr, op0=Alu.mult,
                                            op1=Alu.add)
                    post_ln_affine(nc, tmp, "gfbf")
                    nc.scalar.activation(out=h_t, in_=tmp, func=Act.Gelu)
                else:
                    # fused LN + gelu: gelu(x*rstd + (-mu*rstd))
                    nc.scalar.activation(out=h_t, in_=hpre, func=Act.Gelu,
                                         bias=nmr, scale=rstd)
                nc.sync.dma_start(out=d_h.ap()[t * P:(t + 1) * P, :], in_=h_t)

            for qb in range(NB):
                hT = pc.tile([P, HC, QB], mm_dt, tag="hT", bufs=1)
                for hc in range(HC):
                    nc.sync.dma_start(
                        out=hT[:, hc, :],
                        in_=d_h.ap()[qb * QB:(qb + 1) * QB,
                                     hc * P:(hc + 1) * P],
                        transpose=True)
                for s in range(SB):
                    qs = qb * SB + s
                    ps = psF.tile([P, D], F32, tag="psF")
                    for lo, hi in D_SL:
                        for hc in range(HC):
                            nc.tensor.matmul(
                                ps[:, lo:hi], lhsT=hT[:, hc, s * P:(s + 1) * P],
                                rhs=w2[:, hc, lo:hi], start=(hc == 0),
                                stop=(hc == HC - 1 and
                                      "bf2" not in nontrivial))
                        if "bf2" in nontrivial:
                            add_bias_row(ps, "bf2", lo, hi)
                    o_t = pc.tile([P, D], F32, tag="oldout", bufs=3)
                    nc.sync.dma_start(out=o_t,
                                      in_=d_outf.ap()[qs * P:(qs + 1) * P, :])
                    r2 = pc.tile([P, D], F32, tag="r2", bufs=2)
                    nc.vector.tensor_add(r2, ps, o_t)
                    st3 = pc.tile([P, bn_dn, 6], F32, tag="st3", bufs=2)
                    for g in range(bn_dn):
                        nc.vector.bn_stats(st3[:, g, :],
                                           r2[:, g * bn_d:(g + 1) * bn_d])
                    mv3 = pc.tile([P, 2], F32, tag="mv3", bufs=2)
                    nc.vector.bn_aggr(mv3, st3)
                    rstd3 = emit_rsqrt(pc, nc, mv3[:, 1:2], "ln3")
                    nmr3 = pc.tile([P, 1], F32, tag="nmr3", bufs=2)
                    nc.vector.tensor_scalar(out=nmr3, in0=mv3[:, 0:1],
                                            scalar1=rstd3, scalar2=-1.0,
                                            op0=Alu.mult, op1=Alu.mult)
                    y_t = pc.tile([P, D], F32, tag="y", bufs=3)
                    nc.vector.tensor_scalar(out=y_t, in0=r2, scalar1=rstd3,
                                            scalar2=nmr3, op0=Alu.mult,
                                            op1=Alu.add)
                    post_ln_affine(nc, y_t, "g2b2")
                    nc.sync.dma_start(out=d_y.ap()[qs * P:(qs + 1) * P, :],
                                      in_=y_t)

    nc.compile()
    return nc


# ---------------------------------------------------------------------------
# SPMD runner (jit once, device-resident buffers)
# ---------------------------------------------------------------------------

class SpmdRunner:
    def __init__(self, nc, n_cores=8):
        import jax
        from jax.sharding import Mesh, PartitionSpec, NamedSharding
        from jax.experimental.shard_map import shard_map
        bass2jax.install_neuronx_cc_hook()
        self.jax = jax
        self.nc = nc
        self.n_cores = n_cores
        in_names, out_names, out_avals, zero_outs = [], [], [], []
        part = nc.partition_id_tensor.name if nc.partition_id_tensor else None
        for alloc in nc.m.functions[0].allocations:
            if not isinstance(alloc, mybir.MemoryLocationSet):
                continue
            name = alloc.memorylocations[0].name
            if alloc.kind == "ExternalInput":
                if name != part:
                    in_names.append(name)
            elif alloc.kind == "ExternalOutput":
                out_names.append(name)
                shape = tuple(alloc.tensor_shape)
                dtype = mybir.dt.np(alloc.dtype)
                out_avals.append(jax.core.ShapedArray(shape, dtype))
                zero_outs.append(np.zeros(shape, dtype))
        self.in_names = in_names
        self.out_names = out_names
        self.out_avals = out_avals
        self.zero_outs = zero_outs
        n_params = len(in_names)
        all_names = in_names + out_names + ([part] if part else [])

        def _body(*args):
            operands = list(args)
            if part is not None:
                operands.append(bass2jax.partition_id_tensor())
            return tuple(bass2jax._bass_exec_p.bind(
                *operands, out_avals=tuple(out_avals),
                in_names=tuple(all_names), out_names=tuple(out_names),
                lowering_input_output_aliases=(),
                sim_require_finite=True, sim_require_nnan=True, nc=nc))

        devices = jax.devices()[:n_cores]
        self.mesh = Mesh(np.asarray(devices), ("core",))
        in_specs = (PartitionSpec("core"),) * (n_params + len(out_names))
        out_specs = (PartitionSpec("core"),) * len(out_names)
        self.fn = jax.jit(
            shard_map(_body, mesh=self.mesh, in_specs=in_specs,
                      out_specs=out_specs, check_rep=False),
            keep_unused=True)
        self.sharding = NamedSharding(self.mesh, PartitionSpec("core"))

    def put_inputs(self, in_maps):
        concat = [np.concatenate([np.asarray(in_maps[c][n])
                                  for c in range(self.n_cores)], axis=0)
                  for n in self.in_names]
        zeros = [np.zeros((self.n_cores * z.shape[0], *z.shape[1:]), z.dtype)
                 for z in self.zero_outs]
        bufs = [self.jax.device_put(a, self.sharding) for a in concat + zeros]
        self.jax.block_until_ready(bufs)
        return bufs

    def run(self, bufs):
        outs = self.fn(*bufs)
        self.jax.block_until_ready(outs)
        return outs

    def results(self, outs):
        res = []
        for c in range(self.n_cores):
            d = {}
            for i, name in enumerate(self.out_names):
                d[name] = np.asarray(outs[i]).reshape(
                    self.n_cores, *self.out_avals[i].shape)[c]
            res.append(d)
        return res


# ---------------------------------------------------------------------------
# host entry point
# ---------------------------------------------------------------------------

_CACHE = {}


def _get_runner(nontrivial):
    key = frozenset(nontrivial)
    if key not in _CACHE:
        nc = build_program(nontrivial=key)
        _CACHE[key] = SpmdRunner(nc, 8)
    return _CACHE[key]


def _bf16(a):
    return np.asarray(a, dtype=ml_dtypes.bfloat16)


def kernel(query_modal, key_modal, Wq, bq, Wk, bk, Wv, bv, Wo, bo,
           g1, b1, W1, bf1, gf, bf, W2, bf2, g2, b2):
    query_modal = np.asarray(query_modal, np.float32)
    key_modal = np.asarray(key_modal, np.float32)
    B, N, D = query_modal.shape

    nontrivial = set()
    for nm, val, ident in (("bv", bv, 0.0), ("bo", bo, 0.0),
                           ("bf1", bf1, 0.0), ("bf2", bf2, 0.0)):
        if not np.allclose(np.asarray(val), ident):
            nontrivial.add(nm)
    for nm, g_, b_ in (("g1b1", g1, b1), ("gfbf", gf, bf), ("g2b2", g2, b2)):
        if not (np.allclose(np.asarray(g_), 1.0) and
                np.allclose(np.asarray(b_), 0.0)):
            nontrivial.add(nm)

    runner = _get_runner(nontrivial)

    weights = {
        "wq": _bf16(Wq), "wk": _bf16(Wk), "wv": _bf16(Wv), "wo": _bf16(Wo),
        "w1": _bf16(W1), "w2": _bf16(W2),
        "bq": np.asarray(bq, np.float32), "bk": np.asarray(bk, np.float32),
    }
    for nm, val in (("bv", bv), ("bo", bo), ("bf1", bf1), ("bf2", bf2)):
        if nm in nontrivial:
            weights[nm] = _bf16(val)
    for nm, g_, b_ in (("g1b1", g1, b1), ("gfbf", gf, bf), ("g2b2", g2, b2)):
        if nm in nontrivial:
            weights[nm + "_g"] = np.asarray(g_, np.float32)
            weights[nm + "_b"] = np.asarray(b_, np.float32)

    n_cores = 8
    runs = (B + n_cores - 1) // n_cores
    y = np.empty((B, N, D), np.float32)
    for r in range(runs):
        in_maps = []
        for c in range(n_cores):
            b = r * n_cores + c
            xq = query_modal[b]
            xk = key_modal[b]
            m = dict(weights)
            m["xqT"] = _bf16(np.ascontiguousarray(xq.T))
            m["xkT"] = _bf16(np.ascontiguousarray(xk.T))
            m["xq"] = xq
            in_maps.append(m)
        bufs = runner.put_inputs(in_maps)
        outs = runner.run(bufs)
        res = runner.results(outs)
        for c in range(n_cores):
            y[r * n_cores + c] = res[c]["y"]
    return y


# revision 14
# speedup vs baseline: 1.3144x; 1.3144x over previous
"""Trainium2 Bass kernel for CrossModalAttentionImproved.

Single-head cross attention + FFN transformer block:
  q = Xq@Wq+bq; k = Xk@Wk+bk; v = Xk@Wv+bv
  attn = softmax(q k^T / sqrt(D)); ctx = attn@v
  out = LN(Xq + ctx@Wo + bo; g1,b1)
  h = gelu(LN(out@W1 + bf1; gf,bf))
  y = LN(out + h@W2 + bf2; g2,b2)

Sharding: data-parallel over batch. B=16 across 8 cores x 2 sequential
NEFF runs (one batch element per core per run). Params replicated.

Layout strategy (all matmuls bf16, fp32 PSUM accumulate):
  - host pre-transposes Xq/Xk to [D,N] bf16 so projections contract d on
    partitions with no on-chip input transpose
  - QT/KT produced transposed [D,N]; V natural [N,D] with a ones column
  - scoresT[k,q] = KT.T@QT per k-tile; exp on ACT (scale=1/sqrt(D) folded)
  - ctx[q,d+1] = eT.T @ [V|1]: softmax sums land as a per-partition column
    -> reciprocal + tensor_scalar normalize at PSUM eviction
  - ctx PE-transposed -> Wo -> residual+LN (rsqrt via DVE bit-trick Newton,
    keeps ACT tables to {exp, gelu} only)
  - FFN1 natural out; LN+GELU fused into a single ACT op (scale/bias)
  - h bounced through DRAM with DMA-transpose (2-byte xbar) for FFN2
"""

import sys

if '/opt/trn_rl_repo' not in sys.path:
    sys.path.insert(0, '/opt/trn_rl_repo')

import math
from contextlib import ExitStack

import numpy as np
import ml_dtypes

import concourse.bass as bass
import concourse.tile as tile
from concourse import bacc, mybir
from concourse import bass2jax
from concourse.masks import make_identity

F32 = mybir.dt.float32
BF16 = mybir.dt.bfloat16
U32 = mybir.dt.uint32
Alu = mybir.AluOpType
Act = mybir.ActivationFunctionType

EPS = 1e-5
P = 128


# ---------------------------------------------------------------------------
# device program
# ---------------------------------------------------------------------------

def build_program(N=2048, D=768, H=3072, QB=512,
                  nontrivial=frozenset(), mm_dt=BF16):
    """Build + compile the per-core single-batch-element program.

    nontrivial: subset of {bv, bo, bf1, bf2, g1b1, gfbf, g2b2} naming the
    affine params that are not identity and need real ops emitted.
    """
    DC = D // P          # d chunks (6)
    HC = H // P          # h chunks (24)
    RT = N // P          # row tiles (16)
    NB = N // QB         # q blocks (4)
    SB = QB // P         # subtiles per block (4)
    F1N = min(512, H)    # FFN1 n-chunk width
    F1C = H // F1N       # FFN1 n-chunks (6)
    scale = 1.0 / math.sqrt(D)

    def slices(total):
        """Split [0,total) into <=512-wide psum-bank-sized slices."""
        out, lo = [], 0
        while lo < total:
            hi = min(lo + 512, total)
            out.append((lo, hi))
            lo = hi
        return out

    D_SL = slices(D)          # [(0,512),(512,768)]
    D1_SL = slices(D + 1)     # [(0,512),(512,769)]

    nc = bacc.Bacc("TRN2", target_bir_lowering=False, debug=False,
                   num_devices=8)

    # ---- DRAM I/O -----------------------------------------------------
    d_xqT = nc.dram_tensor("xqT", [D, N], mm_dt, kind="ExternalInput")
    d_xkT = nc.dram_tensor("xkT", [D, N], mm_dt, kind="ExternalInput")
    d_xq = nc.dram_tensor("xq", [N, D], F32, kind="ExternalInput")
    d_wq = nc.dram_tensor("wq", [D, D], mm_dt, kind="ExternalInput")
    d_wk = nc.dram_tensor("wk", [D, D], mm_dt, kind="ExternalInput")
    d_wv = nc.dram_tensor("wv", [D, D], mm_dt, kind="ExternalInput")
    d_wo = nc.dram_tensor("wo", [D, D], mm_dt, kind="ExternalInput")
    d_w1 = nc.dram_tensor("w1", [D, H], mm_dt, kind="ExternalInput")
    d_w2 = nc.dram_tensor("w2", [H, D], mm_dt, kind="ExternalInput")
    d_bq = nc.dram_tensor("bq", [D], F32, kind="ExternalInput")
    d_bk = nc.dram_tensor("bk", [D], F32, kind="ExternalInput")
    dram_aff = {}
    for nm, sz in (("bv", D), ("bo", D), ("bf1", H), ("bf2", D)):
        if nm in nontrivial:
            dram_aff[nm] = nc.dram_tensor(nm, [sz], mm_dt, kind="ExternalInput")
    for nm, sz in (("g1b1", D), ("gfbf", H), ("g2b2", D)):
        if nm in nontrivial:
            dram_aff[nm + "_g"] = nc.dram_tensor(nm + "_g", [sz], F32,
                                                 kind="ExternalInput")
            dram_aff[nm + "_b"] = nc.dram_tensor(nm + "_b", [sz], F32,
                                                 kind="ExternalInput")
    d_y = nc.dram_tensor("y", [N, D], F32, kind="ExternalOutput")
    # internal scratch
    d_outf = nc.dram_tensor("out_f32", [N, D], F32)
    d_outb = nc.dram_tensor("out_b16", [N, D], mm_dt)
    d_h = nc.dram_tensor("h_b16", [N, H], mm_dt)

    # bn_stats subgroup sizes
    bn_d = math.gcd(512, D)      # 256 for 768
    bn_dn = D // bn_d

    def emit_rsqrt(pool, nc, var_ap, tag):
        """rstd[P,1] f32 = 1/sqrt(var+EPS), DVE only (no ACT tables).

        Quake-style bit trick seed + 2 Newton iterations (~5e-6 rel err).
        """
        ve = pool.tile([P, 1], F32, tag=f"rs_ve_{tag}", bufs=2)
        nc.vector.tensor_scalar_add(ve, var_ap, EPS)
        y = pool.tile([P, 1], F32, tag=f"rs_y_{tag}", bufs=2)
        # y_bits = 0x5f3759df - (ve_bits >> 1)  ==  ~(ve_bits>>1) - 0xA0C8A620
        nc.vector.tensor_scalar(
            out=y.bitcast(U32), in0=ve.bitcast(U32),
            scalar1=1, scalar2=0xFFFFFFFF,
            op0=Alu.logical_shift_right, op1=Alu.bitwise_xor)
        nc.vector.tensor_scalar(
            out=y.bitcast(U32), in0=y.bitcast(U32),
            scalar1=0xA0C8A620, scalar2=None, op0=Alu.subtract)
        t = pool.tile([P, 1], F32, tag=f"rs_t_{tag}", bufs=2)
        for _ in range(2):
            nc.vector.tensor_mul(t, y, y)            # y^2
            nc.vector.tensor_mul(t, t, ve)           # v*y^2
            nc.vector.tensor_scalar(out=t, in0=t, scalar1=-0.5, scalar2=1.5,
                                    op0=Alu.mult, op1=Alu.add)
            nc.vector.tensor_mul(y, y, t)
        return y

    def load_rep(pool, nc, dram, sz, tag):
        """Broadcast a [sz] dram vector across partitions -> [P, sz] tile."""
        t = pool.tile([P, sz], dram.dtype, tag=tag, bufs=1)
        ap = dram.ap()
        bcast = bass.AP(tensor=ap.tensor, offset=ap.offset,
                        ap=[[0, P]] + list(ap.ap))
        nc.gpsimd.dma_start(out=t, in_=bcast)
        return t

    with tile.TileContext(nc) as tc, ExitStack() as octx:
        pers = octx.enter_context(tc.tile_pool(name="pers", bufs=1))
        ident = pers.tile([P, P], mm_dt)
        make_identity(nc, ident)
        # per-dout-chunk bias columns for QT/KT eviction (free via ACT bias)
        bq_sb = pers.tile([P, DC], F32)
        nc.sync.dma_start(out=bq_sb, in_=d_bq.ap().rearrange("(c p) -> p c", p=P))
        bk_sb = pers.tile([P, DC], F32)
        nc.sync.dma_start(out=bk_sb, in_=d_bk.ap().rearrange("(c p) -> p c", p=P))
        ones_row = None
        if any(k in nontrivial for k in ("bv", "bo", "bf1", "bf2")):
            ones_row = pers.tile([1, P], mm_dt)
            nc.vector.memset(ones_row, 1.0)
        bias_rows = {}
        for nm in ("bv", "bo", "bf1", "bf2"):
            if nm in nontrivial:
                sz = H if nm == "bf1" else D
                t = pers.tile([1, sz], mm_dt, tag=f"brow_{nm}")
                nc.sync.dma_start(out=t, in_=dram_aff[nm].ap().rearrange(
                    "(o f) -> o f", o=1))
                bias_rows[nm] = t
        gain_reps = {}
        for nm in ("g1b1", "gfbf", "g2b2"):
            if nm in nontrivial:
                sz = H if nm == "gfbf" else D
                gain_reps[nm + "_g"] = load_rep(pers, nc, dram_aff[nm + "_g"],
                                                sz, f"grep_{nm}")
                gain_reps[nm + "_b"] = load_rep(pers, nc, dram_aff[nm + "_b"],
                                                sz, f"brep_{nm}")

        def add_bias_row(psum_ap, nm, lo, hi):
            """Accumulate broadcast bias row into psum via K=1 matmul."""
            nc.tensor.matmul(psum_ap[:, lo:hi], lhsT=ones_row,
                             rhs=bias_rows[nm][:, lo:hi],
                             start=False, stop=True)

        def post_ln_affine(nc, buf, nm):
            if nm in nontrivial:
                nc.vector.tensor_mul(buf, buf, gain_reps[nm + "_g"])
                nc.vector.tensor_add(buf, buf, gain_reps[nm + "_b"])

        # ==== cross-phase pool: outT (B->C) + W1 (prefetched early) ====
        crossBC = octx.enter_context(tc.tile_pool(name="crossBC", bufs=1))
        outT = crossBC.tile([P, DC, N], mm_dt)
        w1 = crossBC.tile([P, DC, H], mm_dt)

        # =============== Phases A+B share the Q/K/V tiles ===============
        ab_ctx = octx.enter_context(ExitStack())
        qkv = ab_ctx.enter_context(tc.tile_pool(name="qkv", bufs=1))
        qT = qkv.tile([P, DC, N], mm_dt)
        kT = qkv.tile([P, DC, N], mm_dt)
        v = qkv.tile([P, RT, D + 8], mm_dt)

        with ExitStack() as ctx:
            pa = ctx.enter_context(tc.tile_pool(name="pa", bufs=1))
            psA = ctx.enter_context(tc.tile_pool(name="psA", bufs=3,
                                                 space="PSUM"))
            psV = ctx.enter_context(tc.tile_pool(name="psV", bufs=2,
                                                 space="PSUM"))
            wq = pa.tile([P, DC, D], mm_dt)
            wk = pa.tile([P, DC, D], mm_dt)
            wv = pa.tile([P, DC, D], mm_dt)
            NRB = N // QB

            def load_xc(dram, tag):
                t = pa.tile([P, DC, QB], mm_dt, tag=tag, bufs=3)
                nc.sync.dma_start(out=t, in_=dram.ap().rearrange(
                    "(c p) n -> p c n", p=P)[:, :, rb * QB:(rb + 1) * QB])
                return t

            # DMA issue order = first-need order: wq, xq0, wk, xk0, wv, W1
            rb = 0
            for k in range(DC):
                nc.sync.dma_start(out=wq[:, k, :], in_=d_wq.ap().rearrange(
                    "(c p) f -> p c f", p=P)[:, k, :])
            xq_rbs = [load_xc(d_xqT, "xqTc")]
            for k in range(DC):
                nc.sync.dma_start(out=wk[:, k, :], in_=d_wk.ap().rearrange(
                    "(c p) f -> p c f", p=P)[:, k, :])
            xk_rbs = [load_xc(d_xkT, "xkTc")]
            for k in range(DC):
                nc.sync.dma_start(out=wv[:, k, :], in_=d_wv.ap().rearrange(
                    "(c p) f -> p c f", p=P)[:, k, :])
            # stream xqT/xkT per row-block; QT/KT per (rb, m); V per row tile
            for rb in range(NRB):
                if rb > 0:
                    xq_rbs.append(load_xc(d_xqT, "xqTc"))
                    xk_rbs.append(load_xc(d_xkT, "xkTc"))
                xqTc = xq_rbs[rb]
                xkTc = xk_rbs[rb]
                for dst, w_sb, b_sb, xT in ((qT, wq, bq_sb, xqTc),
                                            (kT, wk, bk_sb, xkTc)):
                    for m in range(DC):
                        ps = psA.tile([P, QB], F32, tag="psA")
                        for k in range(DC):
                            nc.tensor.matmul(
                                ps, lhsT=w_sb[:, k, m * P:(m + 1) * P],
                                rhs=xT[:, k, :],
                                start=(k == 0), stop=(k == DC - 1))
                        nc.scalar.activation(
                            out=dst[:, m, rb * QB:(rb + 1) * QB], in_=ps,
                            func=Act.Identity, bias=b_sb[:, m:m + 1])
                # V for the row tiles of this block
                for st_ in range(QB // P):
                    rt = rb * (QB // P) + st_
                    ps = psV.tile([P, D], F32, tag="psV")
                    for lo, hi in D_SL:
                        for k in range(DC):
                            nc.tensor.matmul(
                                ps[:, lo:hi],
                                lhsT=xkTc[:, k, st_ * P:(st_ + 1) * P],
                                rhs=wv[:, k, lo:hi],
                                start=(k == 0),
                                stop=(k == DC - 1 and "bv" not in nontrivial))
                        if "bv" in nontrivial:
                            add_bias_row(ps, "bv", lo, hi)
                    nc.scalar.copy(v[:, rt, :D], ps)
                    nc.vector.memset(v[:, rt, D:D + 1], 1.0)

            for k in range(DC):
                nc.sync.dma_start(out=w1[:, k, :], in_=d_w1.ap().rearrange(
                    "(c p) f -> p c f", p=P)[:, k, :])

        # =============== Phase B: attention + LN1 =======================
        with ExitStack() as ctx:
            pb = ctx.enter_context(tc.tile_pool(name="pb", bufs=1))
            psS = ctx.enter_context(tc.tile_pool(name="psS", bufs=2,
                                                 space="PSUM"))
            psC = ctx.enter_context(tc.tile_pool(name="psC", bufs=1,
                                                 space="PSUM"))
            psT = ctx.enter_context(tc.tile_pool(name="psT", bufs=1,
                                                 space="PSUM"))
            psW = ctx.enter_context(tc.tile_pool(name="psW", bufs=1,
                                                 space="PSUM"))
            wo = pb.tile([P, DC, D], mm_dt)
            nc.sync.dma_start(out=wo, in_=d_wo.ap().rearrange(
                "(c p) f -> p c f", p=P))

            for qb in range(NB):
                eT = pb.tile([P, RT, QB], mm_dt, tag="eT", bufs=2)
                for kt in range(RT):
                    ps = psS.tile([P, QB], F32, tag="psS")
                    for k in range(DC):
                        nc.tensor.matmul(
                            ps, lhsT=kT[:, k, kt * P:(kt + 1) * P],
                            rhs=qT[:, k, qb * QB:(qb + 1) * QB],
                            start=(k == 0), stop=(k == DC - 1))
                    nc.scalar.activation(out=eT[:, kt, :], in_=ps,
                                         func=Act.Exp, scale=scale)
                for s in range(SB):
                    qs = qb * SB + s          # global q subtile
                    ps = psC.tile([P, D + 1], F32, tag="psC")
                    for lo, hi in D1_SL:
                        for kt in range(RT):
                            nc.tensor.matmul(
                                ps[:, lo:hi],
                                lhsT=eT[:, kt, s * P:(s + 1) * P],
                                rhs=v[:, kt, lo:hi],
                                start=(kt == 0), stop=(kt == RT - 1))
                    recip = pb.tile([P, 1], F32, tag="recip", bufs=2)
                    nc.vector.reciprocal(recip, ps[:, D:D + 1])
                    ctxt = pb.tile([P, D], mm_dt, tag="ctx", bufs=2)
                    nc.vector.tensor_scalar_mul(ctxt, ps[:, :D], recip)
                    ctxT = pb.tile([P, DC, P], mm_dt, tag="ctxT", bufs=2)
                    for j in range(DC):
                        pst = psT.tile([P, P], mm_dt, tag="psT")
                        nc.tensor.transpose(pst, ctxt[:, j * P:(j + 1) * P],
                                            ident)
                        nc.scalar.copy(ctxT[:, j, :], pst)
                    ps_a = psW.tile([P, D], F32, tag="psW")
                    for lo, hi in D_SL:
                        for j in range(DC):
                            nc.tensor.matmul(
                                ps_a[:, lo:hi], lhsT=ctxT[:, j, :],
                                rhs=wo[:, j, lo:hi], start=(j == 0),
                                stop=(j == DC - 1 and "bo" not in nontrivial))
                        if "bo" in nontrivial:
                            add_bias_row(ps_a, "bo", lo, hi)
                    xq_t = pb.tile([P, D], F32, tag="xq", bufs=3)
                    nc.sync.dma_start(out=xq_t,
                                      in_=d_xq.ap()[qs * P:(qs + 1) * P, :])
                    r_t = pb.tile([P, D], F32, tag="r", bufs=2)
                    nc.vector.tensor_add(r_t, ps_a, xq_t)
                    st = pb.tile([P, bn_dn, 6], F32, tag="st1", bufs=2)
                    for g in range(bn_dn):
                        nc.vector.bn_stats(st[:, g, :],
                                           r_t[:, g * bn_d:(g + 1) * bn_d])
                    mv = pb.tile([P, 2], F32, tag="mv1", bufs=2)
                    nc.vector.bn_aggr(mv, st)
                    rstd = emit_rsqrt(pb, nc, mv[:, 1:2], "ln1")
                    nmr = pb.tile([P, 1], F32, tag="nmr1", bufs=2)
                    nc.vector.tensor_scalar(out=nmr, in0=mv[:, 0:1],
                                            scalar1=rstd, scalar2=-1.0,
                                            op0=Alu.mult, op1=Alu.mult)
                    out_t = pb.tile([P, D], F32, tag="out", bufs=2)
                    nc.vector.tensor_scalar(out=out_t, in0=r_t, scalar1=rstd,
                                            scalar2=nmr, op0=Alu.mult,
                                            op1=Alu.add)
                    post_ln_affine(nc, out_t, "g1b1")
                    nc.sync.dma_start(out=d_outf.ap()[qs * P:(qs + 1) * P, :],
                                      in_=out_t)
                    # cast f32 -> bf16 in flight (SWDGE)
                    nc.gpsimd.dma_start(
                        out=d_outb.ap()[qs * P:(qs + 1) * P, :], in_=out_t)
                # transpose this q-block of `out` back into SBUF for FFN1
                for j in range(DC):
                    nc.sync.dma_start(
                        out=outT[:, j, qb * QB:(qb + 1) * QB],
                        in_=d_outb.ap()[qb * QB:(qb + 1) * QB,
                                        j * P:(j + 1) * P],
                        transpose=True)

        ab_ctx.close()  # free Q/K/V + attention SBUF before FFN

        # =============== Phase C: FFN + LN2/LN3 =========================
        with ExitStack() as ctx:
            pc = ctx.enter_context(tc.tile_pool(name="pc", bufs=1))
            psH = ctx.enter_context(tc.tile_pool(name="psH", bufs=3,
                                                 space="PSUM"))
            psF = ctx.enter_context(tc.tile_pool(name="psF", bufs=2,
                                                 space="PSUM"))
            w2 = pc.tile([P, HC, D], mm_dt)
            nc.sync.dma_start(out=w2, in_=d_w2.ap().rearrange(
                "(c p) f -> p c f", p=P))

            for t in range(RT):
                hpre = pc.tile([P, H], mm_dt, tag="hpre", bufs=2)
                st = pc.tile([P, F1C, 6], F32, tag="st2", bufs=2)
                for n in range(F1C):
                    ps = psH.tile([P, F1N], F32, tag="psH")
                    for j in range(DC):
                        nc.tensor.matmul(
                            ps, lhsT=outT[:, j, t * P:(t + 1) * P],
                            rhs=w1[:, j, n * F1N:(n + 1) * F1N],
                            start=(j == 0),
                            stop=(j == DC - 1 and "bf1" not in nontrivial))
                    if "bf1" in nontrivial:
                        add_bias_row(ps, "bf1", n * F1N, (n + 1) * F1N)
                    nc.vector.tensor_copy(out=hpre[:, n * F1N:(n + 1) * F1N],
                                          in_=ps)
                    nc.vector.bn_stats(st[:, n, :],
                                       hpre[:, n * F1N:(n + 1) * F1N])
                mv = pc.tile([P, 2], F32, tag="mv2", bufs=2)
                nc.vector.bn_aggr(mv, st)
                rstd = emit_rsqrt(pc, nc, mv[:, 1:2], "ln2")
                nmr = pc.tile([P, 1], F32, tag="nmr2", bufs=2)
                nc.vector.tensor_scalar(out=nmr, in0=mv[:, 0:1], scalar1=rstd,
                                        scalar2=-1.0, op0=Alu.mult,
                                        op1=Alu.mult)
                h_t = pc.tile([P, H], mm_dt, tag="h", bufs=2)
                if "gfbf" in nontrivial:
                    tmp = pc.tile([P, H], F32, tag="lnh", bufs=2)
                    nc.vector.tensor_scalar(out=tmp, in0=hpre, scalar1=rstd,
                                            scalar2=nmr, op0=Alu.mult,
                                            op1=Alu.add)
                    post_ln_affine(nc, tmp, "gfbf")
                    nc.scalar.activation(out=h_t, in_=tmp, func=Act.Gelu)
                else:
                    # fused LN + gelu: gelu(x*rstd + (-mu*rstd))
                    nc.scalar.activation(out=h_t, in_=hpre, func=Act.Gelu,
                                         bias=nmr, scale=rstd)
                nc.sync.dma_start(out=d_h.ap()[t * P:(t + 1) * P, :], in_=h_t)

            for qb in range(NB):
                hT = pc.tile([P, HC, QB], mm_dt, tag="hT", bufs=2)
                for hc in range(HC):
                    nc.sync.dma_start(
                        out=hT[:, hc, :],
                        in_=d_h.ap()[qb * QB:(qb + 1) * QB,
                                     hc * P:(hc + 1) * P],
                        transpose=True)
                for s in range(SB):
                    qs = qb * SB + s
                    ps = psF.tile([P, D], F32, tag="psF")
                    for lo, hi in D_SL:
                        for hc in range(HC):
                            nc.tensor.matmul(
                                ps[:, lo:hi], lhsT=hT[:, hc, s * P:(s + 1) * P],
                                rhs=w2[:, hc, lo:hi], start=(hc == 0),
                                stop=(hc == HC - 1 and
                                      "bf2" not in nontrivial))
                        if "bf2" in nontrivial:
                            add_bias_row(ps, "bf2", lo, hi)
                    o_t = pc.tile([P, D], F32, tag="oldout", bufs=3)
                    nc.sync.dma_start(out=o_t,
                                      in_=d_outf.ap()[qs * P:(qs + 1) * P, :])
                    r2 = pc.tile([P, D], F32, tag="r2", bufs=2)
                    nc.vector.tensor_add(r2, ps, o_t)
                    st3 = pc.tile([P, bn_dn, 6], F32, tag="st3", bufs=2)
                    for g in range(bn_dn):
                        nc.vector.bn_stats(st3[:, g, :],
                                           r2[:, g * bn_d:(g + 1) * bn_d])
                    mv3 = pc.tile([P, 2], F32, tag="mv3", bufs=2)
                    nc.vector.bn_aggr(mv3, st3)
                    rstd3 = emit_rsqrt(pc, nc, mv3[:, 1:2], "ln3")
                    nmr3 = pc.tile([P, 1], F32, tag="nmr3", bufs=2)
                    nc.vector.tensor_scalar(out=nmr3, in0=mv3[:, 0:1],
                                            scalar1=rstd3, scalar2=-1.0,
                                            op0=Alu.mult, op1=Alu.mult)
                    y_t = pc.tile([P, D], F32, tag="y", bufs=3)
                    nc.vector.tensor_scalar(out=y_t, in0=r2, scalar1=rstd3,
                                            scalar2=nmr3, op0=Alu.mult,
                                            op1=Alu.add)
                    post_ln_affine(nc, y_t, "g2b2")
                    nc.sync.dma_start(out=d_y.ap()[qs * P:(qs + 1) * P, :],
                                      in_=y_t)

    nc.compile()
    return nc


# ---------------------------------------------------------------------------
# SPMD runner (jit once, device-resident buffers)
# ---------------------------------------------------------------------------

class SpmdRunner:
    def __init__(self, nc, n_cores=8):
        import jax
        from jax.sharding import Mesh, PartitionSpec, NamedSharding
        from jax.experimental.shard_map import shard_map
        bass2jax.install_neuronx_cc_hook()
        self.jax = jax
        self.nc = nc
        self.n_cores = n_cores
        in_names, out_names, out_avals, zero_outs = [], [], [], []
        part = nc.partition_id_tensor.name if nc.partition_id_tensor else None
        for alloc in nc.m.functions[0].allocations:
            if not isinstance(alloc, mybir.MemoryLocationSet):
                continue
            name = alloc.memorylocations[0].name
            if alloc.kind == "ExternalInput":
                if name != part:
                    in_names.append(name)
            elif alloc.kind == "ExternalOutput":
                out_names.append(name)
                shape = tuple(alloc.tensor_shape)
                dtype = mybir.dt.np(alloc.dtype)
                out_avals.append(jax.core.ShapedArray(shape, dtype))
                zero_outs.append(np.zeros(shape, dtype))
        self.in_names = in_names
        self.out_names = out_names
        self.out_avals = out_avals
        self.zero_outs = zero_outs
        n_params = len(in_names)
        all_names = in_names + out_names + ([part] if part else [])

        def _body(*args):
            operands = list(args)
            if part is not None:
                operands.append(bass2jax.partition_id_tensor())
            return tuple(bass2jax._bass_exec_p.bind(
                *operands, out_avals=tuple(out_avals),
                in_names=tuple(all_names), out_names=tuple(out_names),
                lowering_input_output_aliases=(),
                sim_require_finite=True, sim_require_nnan=True, nc=nc))

        devices = jax.devices()[:n_cores]
        self.mesh = Mesh(np.asarray(devices), ("core",))
        in_specs = (PartitionSpec("core"),) * (n_params + len(out_names))
        out_specs = (PartitionSpec("core"),) * len(out_names)
        self.fn = jax.jit(
            shard_map(_body, mesh=self.mesh, in_specs=in_specs,
                      out_specs=out_specs, check_rep=False),
            keep_unused=True)
        self.sharding = NamedSharding(self.mesh, PartitionSpec("core"))

    def put_inputs(self, in_maps):
        concat = [np.concatenate([np.asarray(in_maps[c][n])
                                  for c in range(self.n_cores)], axis=0)
                  for n in self.in_names]
        zeros = [np.zeros((self.n_cores * z.shape[0], *z.shape[1:]), z.dtype)
                 for z in self.zero_outs]
        bufs = [self.jax.device_put(a, self.sharding) for a in concat + zeros]
        self.jax.block_until_ready(bufs)
        return bufs

    def run(self, bufs):
        outs = self.fn(*bufs)
        self.jax.block_until_ready(outs)
        return outs

    def results(self, outs):
        res = []
        for c in range(self.n_cores):
            d = {}
            for i, name in enumerate(self.out_names):
                d[name] = np.asarray(outs[i]).reshape(
                    self.n_cores, *self.out_avals[i].shape)[c]
            res.append(d)
        return res


# ---------------------------------------------------------------------------
# host entry point
# ---------------------------------------------------------------------------

_CACHE = {}


def _get_runner(nontrivial):
    key = frozenset(nontrivial)
    if key not in _CACHE:
        nc = build_program(nontrivial=key)
        _CACHE[key] = SpmdRunner(nc, 8)
    return _CACHE[key]


def _bf16(a):
    return np.asarray(a, dtype=ml_dtypes.bfloat16)


def kernel(query_modal, key_modal, Wq, bq, Wk, bk, Wv, bv, Wo, bo,
           g1, b1, W1, bf1, gf, bf, W2, bf2, g2, b2):
    query_modal = np.asarray(query_modal, np.float32)
    key_modal = np.asarray(key_modal, np.float32)
    B, N, D = query_modal.shape

    nontrivial = set()
    for nm, val, ident in (("bv", bv, 0.0), ("bo", bo, 0.0),
                           ("bf1", bf1, 0.0), ("bf2", bf2, 0.0)):
        if not np.allclose(np.asarray(val), ident):
            nontrivial.add(nm)
    for nm, g_, b_ in (("g1b1", g1, b1), ("gfbf", gf, bf), ("g2b2", g2, b2)):
        if not (np.allclose(np.asarray(g_), 1.0) and
                np.allclose(np.asarray(b_), 0.0)):
            nontrivial.add(nm)

    runner = _get_runner(nontrivial)

    weights = {
        "wq": _bf16(Wq), "wk": _bf16(Wk), "wv": _bf16(Wv), "wo": _bf16(Wo),
        "w1": _bf16(W1), "w2": _bf16(W2),
        "bq": np.asarray(bq, np.float32), "bk": np.asarray(bk, np.float32),
    }
    for nm, val in (("bv", bv), ("bo", bo), ("bf1", bf1), ("bf2", bf2)):
        if nm in nontrivial:
            weights[nm] = _bf16(val)
    for nm, g_, b_ in (("g1b1", g1, b1), ("gfbf", gf, bf), ("g2b2", g2, b2)):
        if nm in nontrivial:
            weights[nm + "_g"] = np.asarray(g_, np.float32)
            weights[nm + "_b"] = np.asarray(b_, np.float32)

    n_cores = 8
    runs = (B + n_cores - 1) // n_cores
    y = np.empty((B, N, D), np.float32)
    for r in range(runs):
        in_maps = []
        for c in range(n_cores):
            b = r * n_cores + c
            xq = query_modal[b]
            xk = key_modal[b]
            m = dict(weights)
            m["xqT"] = _bf16(np.ascontiguousarray(xq.T))
            m["xkT"] = _bf16(np.ascontiguousarray(xk.T))
            m["xq"] = xq
            in_maps.append(m)
        bufs = runner.put_inputs(in_maps)
        outs = runner.run(bufs)
        res = runner.results(outs)
        for c in range(n_cores):
            y[r * n_cores + c] = res[c]["y"]
    return y
